# revision 5
# baseline (speedup 1.0000x reference)
"""Trainium2 Bass kernel for nn_DecoderBlock (self-attn + cross-attn + MLP).

Sharding: pure data-parallel over batch (B=8 -> 8 NeuronCores), no
collectives. Per core the whole block runs with activations feature-major
(features on SBUF partitions, tokens on the free axis) so every matmul uses
the natural [in, out] weight layout as lhsT. RoPE's rotate-half is folded
into extra matmuls with column-permuted/negated weight copies (host-prepped);
LayerNorm gamma/beta are folded into the following weights/biases (host).
Softmax runs unnormalized (exp on ScalarE with the 1/sqrt(d) scale folded
in); the per-(head, token) denominator comes from an all-ones column packed
into the V lhsT tile inside the AV matmul, is reciprocal'd via ACT Ln ->
Exp(-x), and partition-broadcast with a DMA bounce through DRAM. Matmuls run
in bf16 (fp32 PSUM accumulation); the residual stream stays fp32.
"""
import numpy as np

DIM, HEADS, HD = 768, 12, 64
N = 1024
B = 8
EPS = 1e-5
FREQ = 100.0
P = 128
C = DIM // P            # 6 feature chunks
NP = HEADS // 2         # 6 head pairs
KC = N // P             # 8 key chunks
T = N // P              # 8 token chunks
HM = (4 * DIM) // P     # 24 hidden chunks
HH = HM // 2            # 12 hidden chunks per MLP half


# ---------------------------------------------------------------- host prep

def _rope_tables(pos2d):
    """pos2d [N,2] int -> cos, sin [64, N] fp32 (y-half then x-half)."""
    j = np.arange(16, dtype=np.float32)
    inv = 1.0 / (FREQ ** (2.0 * j / 32.0))
    n = pos2d.shape[0]
    c = np.empty((64, n), np.float32)
    s = np.empty((64, n), np.float32)
    for half, p in ((0, pos2d[:, 0]), (1, pos2d[:, 1])):
        f = p.astype(np.float32)[None, :] * inv[:, None]
        emb = np.concatenate([f, f], 0)
        c[half * 32:(half + 1) * 32] = np.cos(emb)
        s[half * 32:(half + 1) * 32] = np.sin(emb)
    return c, s


def _rot_weight(w):
    wr = np.empty_like(w)
    for base in range(0, w.shape[1], 32):
        wr[:, base:base + 16] = -w[:, base + 16:base + 32]
        wr[:, base + 16:base + 32] = w[:, base:base + 16]
    return wr


def _rot_bias(b):
    br = np.empty_like(b)
    for base in range(0, b.shape[0], 32):
        br[base:base + 16] = -b[base + 16:base + 32]
        br[base + 16:base + 32] = b[base:base + 16]
    return br


def _feat_major(b):
    return np.ascontiguousarray(b.reshape(-1, P).T.astype(np.float32))


def prep_host(inputs):
    """Returns a list of per-core input dicts (weights shared)."""
    import ml_dtypes
    f32 = np.float32
    inp = {k: np.asarray(v) for k, v in inputs.items()}
    g1, b1 = inp['norm1_g'].astype(f32), inp['norm1_b'].astype(f32)
    g2, b2 = inp['norm2_g'].astype(f32), inp['norm2_b'].astype(f32)
    g3, b3 = inp['norm3_g'].astype(f32), inp['norm3_b'].astype(f32)
    gy, by = inp['normy_g'].astype(f32), inp['normy_b'].astype(f32)
    qkv = inp['qkv_w'].astype(f32)
    wq, wk, wv = qkv[:, :DIM], qkv[:, DIM:2 * DIM], qkv[:, 2 * DIM:]

    def fold(g, b, wmat):
        return (g[:, None] * wmat).astype(f32), (b @ wmat).astype(f32)

    wqA, bq = fold(g1, b1, wq)
    wkA, bk = fold(g1, b1, wk)
    wvF, bv = fold(g1, b1, wv)
    wcqA, bcq = fold(g2, b2, inp['projq_w'].astype(f32))
    wckA, bck = fold(gy, by, inp['projk_w'].astype(f32))
    wcvF, bcv = fold(gy, by, inp['projv_w'].astype(f32))
    wfc1, bfc1x = fold(g3, b3, inp['fc1_w'].astype(f32))
    bfc1 = inp['fc1_b'].astype(f32) + bfc1x

    bf = ml_dtypes.bfloat16
    shared = {
        'wqA': wqA.astype(bf), 'wqB': _rot_weight(wqA).astype(bf),
        'wkA': wkA.astype(bf), 'wkB': _rot_weight(wkA).astype(bf),
        'wv': wvF.astype(bf),
        'wproj': inp['attn_proj_w'].astype(f32).astype(bf),
        'wcqA': wcqA.astype(bf), 'wcqB': _rot_weight(wcqA).astype(bf),
        'wckA': wckA.astype(bf), 'wckB': _rot_weight(wckA).astype(bf),
        'wcv': wcvF.astype(bf),
        'wcproj': inp['cross_proj_w'].astype(f32).astype(bf),
        'wfc1': wfc1.astype(bf),
        'wfc2': inp['fc2_w'].astype(f32).astype(bf),
        'bq': _feat_major(bq), 'bqr': _feat_major(_rot_bias(bq)),
        'bk': _feat_major(bk), 'bkr': _feat_major(_rot_bias(bk)),
        'bcq': _feat_major(bcq), 'bcqr': _feat_major(_rot_bias(bcq)),
        'bck': _feat_major(bck), 'bckr': _feat_major(_rot_bias(bck)),
        'bproj': _feat_major(inp['attn_proj_b'].astype(f32)),
        'bcproj': _feat_major(inp['cross_proj_b'].astype(f32)),
        'bfc1': np.ascontiguousarray(bfc1.reshape(-1, P).T.astype(f32)),
        'bfc2': _feat_major(inp['fc2_b'].astype(f32)),
        'bv_row': bv.reshape(1, DIM).astype(f32),
        'bcv_row': bcv.reshape(1, DIM).astype(f32),
        'ones_bf': np.ones((P, P), bf),
    }
    per_core = []
    for bi in range(B):
        cxx, sxn = _rope_tables(inp['xpos'][bi])
        cyn, syn = _rope_tables(inp['ypos'][bi])
        d = {
            'xT': np.ascontiguousarray(inp['x'][bi].T.astype(f32)),
            'yT': np.ascontiguousarray(inp['y'][bi].T.astype(f32)),
            'cosx': np.ascontiguousarray(np.tile(cxx, (2, 1))),
            'sinx': np.ascontiguousarray(np.tile(sxn, (2, 1))),
            'cosy': np.ascontiguousarray(np.tile(cyn, (2, 1))),
            'siny': np.ascontiguousarray(np.tile(syn, (2, 1))),
        }
        d.update(shared)
        per_core.append(d)
    return per_core


# ------------------------------------------------------- walrus workarounds

def split_excess_waits(nc, max_waits=1):
    """This walrus build rejects instructions carrying more than one
    sync-wait on CTRL-class instructions. Move excess waits onto NoOps
    inserted immediately before the offending instruction on the same
    engine (same-engine program order keeps semantics)."""
    import concourse.mybir as mybir
    n_split = 0
    cnt = [0]
    for f in nc.m.functions:
        for blk in f.blocks:
            insts = list(blk.instructions)
            out = []
            changed = False
            for inst in insts:
                si = inst.sync_info
                waits = list(si.on_wait) if si and si.on_wait else []
                if len(waits) > max_waits:
                    changed = True
                    n_split += 1
                    extra = waits[:-max_waits]
                    keep = waits[-max_waits:]
                    while extra:
                        chunk, extra = extra[:max_waits], extra[max_waits:]
                        cnt[0] += 1
                        nop = mybir.InstNoOp(
                            name=f"WSPLIT-{id(nc) % 100000}-{cnt[0]}",
                            ins=[], outs=[], engine=inst.engine)
                        nop.sync_info = mybir.SyncInfo(on_wait=chunk,
                                                       on_update=[])
                        out.append(nop)
                    inst.sync_info = mybir.SyncInfo(
                        on_wait=keep,
                        on_update=list(si.on_update) if si.on_update else [])
                out.append(inst)
            if changed:
                blk.instructions = out
    return n_split


# ------------------------------------------------------------- kernel build

def build_nc(k_iters=1):
    import concourse.bass as bass
    import concourse.mybir as mybir
    from concourse.tile import TileContext

    F32 = mybir.dt.float32
    BF16 = mybir.dt.bfloat16
    AF = mybir.ActivationFunctionType
    OP = mybir.AluOpType

    nc = bass.Bass()
    d = {}
    for name, shape, dt in [
        ('xT', [DIM, N], F32), ('yT', [DIM, N], F32),
        ('cosx', [P, N], F32), ('sinx', [P, N], F32),
        ('cosy', [P, N], F32), ('siny', [P, N], F32),
        ('wqA', [DIM, DIM], BF16), ('wqB', [DIM, DIM], BF16),
        ('wkA', [DIM, DIM], BF16), ('wkB', [DIM, DIM], BF16),
        ('wv', [DIM, DIM], BF16), ('wproj', [DIM, DIM], BF16),
        ('wcqA', [DIM, DIM], BF16), ('wcqB', [DIM, DIM], BF16),
        ('wckA', [DIM, DIM], BF16), ('wckB', [DIM, DIM], BF16),
        ('wcv', [DIM, DIM], BF16), ('wcproj', [DIM, DIM], BF16),
        ('wfc1', [DIM, 4 * DIM], BF16), ('wfc2', [4 * DIM, DIM], BF16),
        ('bq', [P, C], F32), ('bqr', [P, C], F32),
        ('bk', [P, C], F32), ('bkr', [P, C], F32),
        ('bcq', [P, C], F32), ('bcqr', [P, C], F32),
        ('bck', [P, C], F32), ('bckr', [P, C], F32),
        ('bproj', [P, C], F32), ('bcproj', [P, C], F32),
        ('bfc1', [P, HM], F32), ('bfc2', [P, C], F32),
        ('bv_row', [1, DIM], F32), ('bcv_row', [1, DIM], F32),
        ('ones_bf', [P, P], BF16),
    ]:
        d[name] = nc.declare_dram_parameter(name, shape, dt, isOutput=False)
    out_d = nc.declare_dram_parameter('outT', [DIM, N], F32, isOutput=True)

    with TileContext(nc) as tc:
        with tc.tile_pool(name="const", bufs=1) as const, \
             tc.tile_pool(name="main", bufs=1) as main, \
             tc.tile_pool(name="work", bufs=2) as work, \
             tc.tile_pool(name="dscr", bufs=4, space="DRAM") as dscr, \
             tc.tile_pool(name="ps", bufs=4, space="PSUM") as psp:

            def body():
                # ---- constants ----
                cos_x = const.tile([P, N], F32, tag='cosx', name='cos_x')
                sin_x = const.tile([P, N], F32, tag='sinx', name='sin_x')
                cos_y = const.tile([P, N], F32, tag='cosy', name='cos_y')
                sin_y = const.tile([P, N], F32, tag='siny', name='sin_y')
                nc.sync.dma_start(cos_x[:], d['cosx'][:])
                nc.sync.dma_start(sin_x[:], d['sinx'][:])
                nc.sync.dma_start(cos_y[:], d['cosy'][:])
                nc.sync.dma_start(sin_y[:], d['siny'][:])
                ones_bf = const.tile([P, P], BF16, tag='ones', name='ones_bf')
                nc.sync.dma_start(ones_bf[:], d['ones_bf'][:])
                bias = {}
                for nm in ('bq', 'bqr', 'bk', 'bkr', 'bcq', 'bcqr', 'bck',
                           'bckr', 'bproj', 'bcproj', 'bfc2'):
                    bias[nm] = const.tile([P, C], F32, tag=nm, name=nm)
                    nc.sync.dma_start(bias[nm][:], d[nm][:])
                bias['bfc1'] = const.tile([P, HM], F32, tag='bfc1',
                                          name='bfc1')
                nc.sync.dma_start(bias['bfc1'][:], d['bfc1'][:])
                eps_t = const.tile([P, 1], F32, tag='eps', name='eps_t')
                nc.vector.memset(eps_t[:], EPS)
                bvrep = const.tile([P, DIM], F32, tag='bvrep', name='bvrep')
                nc.sync.dma_start(bvrep[:],
                                  d['bv_row'][:].to_broadcast((P, DIM)))
                bcvrep = const.tile([P, DIM], F32, tag='bcvrep', name='bcvrep')
                nc.sync.dma_start(bcvrep[:],
                                  d['bcv_row'][:].to_broadcast((P, DIM)))

                # ---- residual load ----
                xT = main.tile([P, C, N], F32, tag='xT', name='xT')
                nc.sync.dma_start(
                    xT[:], d['xT'][:].rearrange("(c p) t -> p c t", p=P))

                def w_cols(wd, row0, rcnt, col0, cw, tag='wlhs'):
                    """[rcnt*128, cw] weight block as [128, rcnt, cw] bf16."""
                    t = work.tile([P, rcnt, cw], BF16, tag=tag, bufs=3,
                                  name=f'w_{tag}')
                    nc.sync.dma_start(
                        t[:], wd[row0:row0 + rcnt * P, col0:col0 + cw]
                        .rearrange("(c p) w -> p c w", p=P))
                    return t

                def layernorm(src, dst_tag):
                    """src [128, C, N] f32 -> bf16 normalized tile."""
                    xbf = main.tile([P, C, N], BF16, tag='expSC', name='xbf')
                    xsq = main.tile([P, C, N], BF16, tag='oTB', name='xsq')
                    for cc in range(C):
                        nc.vector.tensor_copy(xbf[:, cc, :], src[:, cc, :])
                        nc.vector.tensor_tensor(
                            xsq[:, cc, :], xbf[:, cc, :], xbf[:, cc, :],
                            OP.mult)
                    ps1 = psp.tile([P, N], F32, tag='ps', name='ps_sum')
                    ps2 = psp.tile([P, N], F32, tag='ps', name='ps_sumsq')
                    for cc in range(C):
                        for qh in range(2):
                            sl = slice(qh * 512, qh * 512 + 512)
                            nc.tensor.matmul(
                                ps1[:, sl], ones_bf[:], xbf[:, cc, sl],
                                start=(cc == 0), stop=(cc == C - 1))
                            nc.tensor.matmul(
                                ps2[:, sl], ones_bf[:], xsq[:, cc, sl],
                                start=(cc == 0), stop=(cc == C - 1))
                    m_rep = main.tile([P, N], F32, tag='s_m', name='m_rep')
                    nc.vector.tensor_scalar_mul(m_rep[:], ps1[:], 1.0 / DIM)
                    var = main.tile([P, N], F32, tag='s_var', name='var')
                    nc.vector.tensor_tensor(var[:], m_rep[:], m_rep[:],
                                            OP.mult)
                    nc.vector.scalar_tensor_tensor(
                        var[:], ps2[:], 1.0 / DIM, var[:],
                        OP.mult, OP.subtract)
                    nc.scalar.activation(var[:], var[:], AF.Ln, bias=eps_t[:])
                    rstd = main.tile([P, N], F32, tag='s_rstd', name='rstd')
                    nc.scalar.activation(rstd[:], var[:], AF.Exp, scale=-0.5)
                    h = main.tile([P, C, N], BF16, tag=dst_tag, name='h_out')
                    for cc in range(C):
                        tmpf = work.tile([P, N], F32, tag='lntmp', bufs=1,
                                         name='lntmp')
                        nc.vector.tensor_tensor(
                            tmpf[:], src[:, cc, :], m_rep[:], OP.subtract)
                        nc.vector.tensor_tensor(
                            h[:, cc, :], tmpf[:], rstd[:], OP.mult)
                    return h

                def build_vt(h, wv_d, bvr):
                    """V+ones lhsT tile [128, T, HEADS, 128] bf16."""
                    vt = main.tile([P, T, HEADS, P], BF16, tag='bigA',
                                   name='vt')
                    nc.gpsimd.memset(vt[:], 0.0)
                    nc.gpsimd.memset(vt[:, :, 0:HEADS:2, 64:65], 1.0)
                    nc.gpsimd.memset(vt[:, :, 1:HEADS:2, 63:64], 1.0)
                    wvt = main.tile([P, C, DIM], BF16, tag='wv_full',
                                    name='wvt')
                    nc.sync.dma_start(
                        wvt[:], wv_d[:].rearrange("(c p) w -> p c w", p=P))
                    for tci in range(T):
                        pv = psp.tile([P, N], F32, tag='ps', name='pv')
                        for cc in range(C):
                            lhs = h[:, cc, tci * P:(tci + 1) * P]
                            nc.tensor.matmul(
                                pv[:, 0:512], lhs, wvt[:, cc, 0:512],
                                start=(cc == 0), stop=(cc == C - 1))
                            nc.tensor.matmul(
                                pv[:, 512:768], lhs, wvt[:, cc, 512:768],
                                start=(cc == 0), stop=(cc == C - 1))
                        pv_h = pv[:, 0:DIM].rearrange("p (h e) -> p h e", e=HD)
                        bv_h = bvr[:].rearrange("p (h e) -> p h e", e=HD)
                        nc.vector.tensor_tensor(
                            vt[:, tci, 0:HEADS:2, 0:64],
                            pv_h[:, 0:HEADS:2, :], bv_h[:, 0:HEADS:2, :],
                            OP.add)
                        nc.vector.tensor_tensor(
                            vt[:, tci, 1:HEADS:2, 64:128],
                            pv_h[:, 1:HEADS:2, :], bv_h[:, 1:HEADS:2, :],
                            OP.add)
                    return vt

                def qk_pair(h_src, wA, wB, bA, bAr, cost, sint, pi, tag):
                    """Roped Q^T or K^T pair-chunk [128, 1024] bf16."""
                    out = work.tile([P, N], BF16, tag=tag, name=f'{tag}_t')
                    wa = w_cols(wA, 0, C, pi * P, P)
                    wb = w_cols(wB, 0, C, pi * P, P)
                    for qh in range(2):
                        sl = slice(qh * 512, qh * 512 + 512)
                        pq = psp.tile([P, N], F32, tag='ps', name='pq')
                        for cc in range(C):
                            nc.tensor.matmul(
                                pq[:, 0:512], wa[:, cc, :], h_src[:, cc, sl],
                                start=(cc == 0), stop=(cc == C - 1))
                            nc.tensor.matmul(
                                pq[:, 512:1024], wb[:, cc, :],
                                h_src[:, cc, sl],
                                start=(cc == 0), stop=(cc == C - 1))
                        t1 = work.tile([P, 512], F32, tag='rtmp1', bufs=1,
                                       name='rt1')
                        t2 = work.tile([P, 512], F32, tag='rtmp2', bufs=1,
                                       name='rt2')
                        nc.vector.scalar_tensor_tensor(
                            t1[:], pq[:, 0:512], bA[:, pi:pi + 1],
                            cost[:, sl], OP.add, OP.mult)
                        nc.vector.scalar_tensor_tensor(
                            t2[:], pq[:, 512:1024], bAr[:, pi:pi + 1],
                            sint[:, sl], OP.add, OP.mult)
                        nc.vector.tensor_tensor(out[:, sl], t1[:], t2[:],
                                                OP.add)
                    return out

                def attention(h_q, h_k, vt, wgts, biases, cos_q, sin_q,
                              cos_k, sin_k, oT):
                    wqa, wqb, wka, wkb = wgts
                    bqa, bqra, bka, bkra = biases
                    for pi in range(NP):
                        qro = qk_pair(h_q, wqa, wqb, bqa, bqra,
                                      cos_q, sin_q, pi, 'qro')
                        kro = qk_pair(h_k, wka, wkb, bka, bkra,
                                      cos_k, sin_k, pi, 'kro')
                        for qh in range(2):
                            qsl = slice(qh * 512, qh * 512 + 512)
                            expS = main.tile([P, KC, 2, 512], BF16,
                                             tag='expSC', name='expS')
                            for kc in range(KC):
                                pss = psp.tile([P, N], F32, tag='ps',
                                               name='pss')
                                ksl = slice(kc * P, (kc + 1) * P)
                                nc.tensor.matmul(
                                    pss[:, 0:512], kro[0:64, ksl],
                                    qro[0:64, qsl], start=True, stop=True)
                                nc.tensor.matmul(
                                    pss[:, 512:1024], kro[64:128, ksl],
                                    qro[64:128, qsl], start=True, stop=True)
                                nc.scalar.activation(
                                    expS[:, kc, :, :], pss[:],
                                    AF.Exp, scale=float(HD) ** -0.5)
                            pav = psp.tile([P, N], F32, tag='ps', name='pav')
                            for kc in range(KC):
                                nc.tensor.matmul(
                                    pav[:, 0:512], vt[:, kc, 2 * pi, :],
                                    expS[:, kc, 0, :],
                                    start=(kc == 0), stop=(kc == KC - 1))
                                nc.tensor.matmul(
                                    pav[:, 512:1024],
                                    vt[:, kc, 2 * pi + 1, :],
                                    expS[:, kc, 1, :],
                                    start=(kc == 0), stop=(kc == KC - 1))
                            # denominators: row 64 (even head), row 63 (odd)
                            tln = main.tile([P, N], F32, tag='s_var',
                                            name='tln')
                            trec = main.tile([P, N], F32, tag='s_rstd',
                                             name='trec')
                            nc.scalar.activation(tln[:], pav[:], AF.Ln)
                            nc.scalar.activation(trec[:], tln[:], AF.Exp,
                                                 scale=-1.0)
                            scr = dscr.tile([2, N], F32, tag='scr',
                                            name='scr')
                            nc.sync.dma_start(scr[:], trec[63:65, :])
                            rep = work.tile([P, N], F32, tag='rep', bufs=1,
                                            name='rep')
                            nc.sync.dma_start(
                                rep[0:64, 0:512],
                                scr[1:2, 0:512].to_broadcast((64, 512)))
                            nc.sync.dma_start(
                                rep[64:128, 512:1024],
                                scr[0:1, 512:1024].to_broadcast((64, 512)))
                            nc.vector.tensor_tensor(
                                oT[0:64, pi, qsl], pav[0:64, 0:512],
                                rep[0:64, 0:512], OP.mult)
                            nc.vector.tensor_tensor(
                                oT[64:128, pi, qsl], pav[64:128, 512:1024],
                                rep[64:128, 512:1024], OP.mult)

                def proj_residual(oT, w_d, b_sb):
                    for m in range(C):
                        pp = psp.tile([P, N], F32, tag='ps', name='pp')
                        wp = w_cols(w_d, 0, C, m * P, P)
                        for cc in range(C):
                            for qh in range(2):
                                sl = slice(qh * 512, qh * 512 + 512)
                                nc.tensor.matmul(
                                    pp[:, sl], wp[:, cc, :], oT[:, cc, sl],
                                    start=(cc == 0), stop=(cc == C - 1))
                        nc.vector.scalar_tensor_tensor(
                            xT[:, m, :], pp[:], b_sb[:, m:m + 1],
                            xT[:, m, :], OP.add, OP.add)

                # ================= self attention =================
                h1 = layernorm(xT, 'h')
                vt = build_vt(h1, d['wv'], bvrep)
                oT1 = main.tile([P, C, N], BF16, tag='oTB', name='oT1')
                attention(h1, h1, vt,
                          (d['wqA'], d['wqB'], d['wkA'], d['wkB']),
                          (bias['bq'], bias['bqr'], bias['bk'], bias['bkr']),
                          cos_x, sin_x, cos_x, sin_x, oT1)
                proj_residual(oT1, d['wproj'], bias['bproj'])

                # ================= cross attention =================
                yT = main.tile([P, C, N], F32, tag='bigA', name='yT')
                nc.sync.dma_start(
                    yT[:], d['yT'][:].rearrange("(c p) t -> p c t", p=P))
                y_ = layernorm(yT, 'y_')
                h2 = layernorm(xT, 'h')
                vtc = build_vt(y_, d['wcv'], bcvrep)
                oT2 = main.tile([P, C, N], BF16, tag='oTB', name='oT2')
                attention(h2, y_, vtc,
                          (d['wcqA'], d['wcqB'], d['wckA'], d['wckB']),
                          (bias['bcq'], bias['bcqr'], bias['bck'],
                           bias['bckr']),
                          cos_x, sin_x, cos_y, sin_y, oT2)
                proj_residual(oT2, d['wcproj'], bias['bcproj'])

                # ================= MLP (two hidden halves) =================
                h3 = layernorm(xT, 'h')
                outacc = main.tile([P, C, N], F32, tag='oTB', name='outacc')
                for half in range(2):
                    hidh = main.tile([P, HH, N], BF16, tag='bigA',
                                     name='hidh')
                    for hj in range(HH):
                        hm = half * HH + hj
                        ph = psp.tile([P, N], F32, tag='ps', name='ph')
                        wf = w_cols(d['wfc1'], 0, C, hm * P, P)
                        for cc in range(C):
                            for qh in range(2):
                                sl = slice(qh * 512, qh * 512 + 512)
                                nc.tensor.matmul(
                                    ph[:, sl], wf[:, cc, :], h3[:, cc, sl],
                                    start=(cc == 0), stop=(cc == C - 1))
                        nc.scalar.activation(
                            hidh[:, hj, :], ph[:], AF.Gelu,
                            bias=bias['bfc1'][:, hm:hm + 1])
                    for m in range(C):
                        po = psp.tile([P, N], F32, tag='ps', name='po')
                        wf2 = w_cols(d['wfc2'], half * HH * P, HH, m * P, P)
                        for kj in range(HH):
                            for qh in range(2):
                                sl = slice(qh * 512, qh * 512 + 512)
                                nc.tensor.matmul(
                                    po[:, sl], wf2[:, kj, :], hidh[:, kj, sl],
                                    start=(kj == 0), stop=(kj == HH - 1))
                        if half == 0:
                            nc.vector.tensor_scalar_add(
                                outacc[:, m, :], po[:], bias['bfc2'][:, m:m + 1])
                        else:
                            nc.vector.tensor_tensor(
                                outacc[:, m, :], outacc[:, m, :], po[:],
                                OP.add)
                for m in range(C):
                    nc.vector.tensor_tensor(
                        xT[:, m, :], xT[:, m, :], outacc[:, m, :], OP.add)

                # ---- store ----
                nc.sync.dma_start(
                    out_d[:].rearrange("(c p) t -> p c t", p=P), xT[:])

            if k_iters > 1:
                with tc.For_i(0, k_iters, 1):
                    body()
            else:
                body()

    split_excess_waits(nc)
    return nc


# ------------------------------------------------------------------ driver

def kernel(**inputs):
    from concourse.bass_utils import run_bass_kernel_spmd
    per_core = prep_host(inputs)
    nc = build_nc(1)
    res = run_bass_kernel_spmd(nc, per_core, core_ids=list(range(B)))
    x_out = np.stack([np.ascontiguousarray(res.results[i]['outT'].T)
                      for i in range(B)])
    y = np.asarray(inputs['y'], dtype=np.float32)
    return (x_out.astype(np.float32), y)


# revision 6
# speedup vs baseline: 2.0059x; 2.0059x over previous
"""Trainium2 Bass kernel for nn_DecoderBlock (self-attn + cross-attn + MLP).

Sharding: pure data-parallel over batch (B=8 -> 8 NeuronCores), no
collectives. Per core the whole block runs with activations feature-major
(features on SBUF partitions, tokens on the free axis) so every matmul uses
the natural [in, out] weight layout as lhsT. RoPE's rotate-half is folded
into extra matmuls with column-permuted/negated weight copies (host-prepped);
LayerNorm gamma/beta are folded into the following weights/biases (host).
Softmax runs unnormalized (exp on ScalarE with the 1/sqrt(d) scale folded
in); the per-(head, token) denominator comes from an all-ones column packed
into the V lhsT tile inside the AV matmul, is reciprocal'd via ACT Ln ->
Exp(-x), and partition-broadcast with a DMA bounce through DRAM. Matmuls run
in bf16 (fp32 PSUM accumulation); the residual stream stays fp32.
"""
import numpy as np

DIM, HEADS, HD = 768, 12, 64
N = 1024
B = 8
EPS = 1e-5
FREQ = 100.0
P = 128
C = DIM // P            # 6 feature chunks
NP = HEADS // 2         # 6 head pairs
KC = N // P             # 8 key chunks
T = N // P              # 8 token chunks
HM = (4 * DIM) // P     # 24 hidden chunks
HH = HM // 2            # 12 hidden chunks per MLP half


# ---------------------------------------------------------------- host prep

def _rope_tables(pos2d):
    """pos2d [N,2] int -> cos, sin [64, N] fp32 (y-half then x-half)."""
    j = np.arange(16, dtype=np.float32)
    inv = 1.0 / (FREQ ** (2.0 * j / 32.0))
    n = pos2d.shape[0]
    c = np.empty((64, n), np.float32)
    s = np.empty((64, n), np.float32)
    for half, p in ((0, pos2d[:, 0]), (1, pos2d[:, 1])):
        f = p.astype(np.float32)[None, :] * inv[:, None]
        emb = np.concatenate([f, f], 0)
        c[half * 32:(half + 1) * 32] = np.cos(emb)
        s[half * 32:(half + 1) * 32] = np.sin(emb)
    return c, s


def _rot_weight(w):
    wr = np.empty_like(w)
    for base in range(0, w.shape[1], 32):
        wr[:, base:base + 16] = -w[:, base + 16:base + 32]
        wr[:, base + 16:base + 32] = w[:, base:base + 16]
    return wr


def _rot_bias(b):
    br = np.empty_like(b)
    for base in range(0, b.shape[0], 32):
        br[base:base + 16] = -b[base + 16:base + 32]
        br[base + 16:base + 32] = b[base:base + 16]
    return br


def _feat_major(b):
    return np.ascontiguousarray(b.reshape(-1, P).T.astype(np.float32))


def prep_host(inputs):
    """Returns a list of per-core input dicts (weights shared)."""
    import ml_dtypes
    f32 = np.float32
    inp = {k: np.asarray(v) for k, v in inputs.items()}
    g1, b1 = inp['norm1_g'].astype(f32), inp['norm1_b'].astype(f32)
    g2, b2 = inp['norm2_g'].astype(f32), inp['norm2_b'].astype(f32)
    g3, b3 = inp['norm3_g'].astype(f32), inp['norm3_b'].astype(f32)
    gy, by = inp['normy_g'].astype(f32), inp['normy_b'].astype(f32)
    qkv = inp['qkv_w'].astype(f32)
    wq, wk, wv = qkv[:, :DIM], qkv[:, DIM:2 * DIM], qkv[:, 2 * DIM:]

    def fold(g, b, wmat):
        return (g[:, None] * wmat).astype(f32), (b @ wmat).astype(f32)

    wqA, bq = fold(g1, b1, wq)
    wkA, bk = fold(g1, b1, wk)
    wvF, bv = fold(g1, b1, wv)
    wcqA, bcq = fold(g2, b2, inp['projq_w'].astype(f32))
    wckA, bck = fold(gy, by, inp['projk_w'].astype(f32))
    wcvF, bcv = fold(gy, by, inp['projv_w'].astype(f32))
    wfc1, bfc1x = fold(g3, b3, inp['fc1_w'].astype(f32))
    bfc1 = inp['fc1_b'].astype(f32) + bfc1x

    bf = ml_dtypes.bfloat16

    def tiled(w):
        rows, cols = w.shape
        cr, ncb = rows // P, cols // P
        return np.ascontiguousarray(
            np.transpose(w.reshape(cr, P, ncb, P), (2, 1, 0, 3)).astype(bf))

    def vfull(w):
        cr = w.shape[0] // P
        return np.ascontiguousarray(
            np.transpose(w.reshape(cr, P, w.shape[1]), (1, 0, 2)).astype(bf))

    shared = {
        'wqA': tiled(wqA), 'wqB': tiled(_rot_weight(wqA)),
        'wkA': tiled(wkA), 'wkB': tiled(_rot_weight(wkA)),
        'wv': vfull(wvF),
        'wproj': tiled(inp['attn_proj_w'].astype(f32)),
        'wcqA': tiled(wcqA), 'wcqB': tiled(_rot_weight(wcqA)),
        'wckA': tiled(wckA), 'wckB': tiled(_rot_weight(wckA)),
        'wcv': vfull(wcvF),
        'wcproj': tiled(inp['cross_proj_w'].astype(f32)),
        'wfc1': tiled(wfc1),
        'wfc2': tiled(inp['fc2_w'].astype(f32)),
        'bq': _feat_major(bq), 'bqr': _feat_major(_rot_bias(bq)),
        'bk': _feat_major(bk), 'bkr': _feat_major(_rot_bias(bk)),
        'bcq': _feat_major(bcq), 'bcqr': _feat_major(_rot_bias(bcq)),
        'bck': _feat_major(bck), 'bckr': _feat_major(_rot_bias(bck)),
        'bproj': _feat_major(inp['attn_proj_b'].astype(f32)),
        'bcproj': _feat_major(inp['cross_proj_b'].astype(f32)),
        'bfc1': np.ascontiguousarray(bfc1.reshape(-1, P).T.astype(f32)),
        'bfc2': _feat_major(inp['fc2_b'].astype(f32)),
        'bv_row': bv.reshape(1, DIM).astype(f32),
        'bcv_row': bcv.reshape(1, DIM).astype(f32),
        'ones_bf': np.ones((P, P), bf),
    }
    per_core = []
    for bi in range(B):
        cxx, sxn = _rope_tables(inp['xpos'][bi])
        cyn, syn = _rope_tables(inp['ypos'][bi])
        d = {
            'xT': np.ascontiguousarray(inp['x'][bi].T.astype(f32)),
            'yT': np.ascontiguousarray(inp['y'][bi].T.astype(f32)),
            'cosx': np.ascontiguousarray(np.tile(cxx, (2, 1))),
            'sinx': np.ascontiguousarray(np.tile(sxn, (2, 1))),
            'cosy': np.ascontiguousarray(np.tile(cyn, (2, 1))),
            'siny': np.ascontiguousarray(np.tile(syn, (2, 1))),
        }
        d.update(shared)
        per_core.append(d)
    return per_core


# ------------------------------------------------------- walrus workarounds

def split_excess_waits(nc, max_waits=1):
    """This walrus build rejects instructions carrying more than one
    sync-wait on CTRL-class instructions. Move excess waits onto NoOps
    inserted immediately before the offending instruction on the same
    engine (same-engine program order keeps semantics)."""
    import concourse.mybir as mybir
    n_split = 0
    cnt = [0]
    for f in nc.m.functions:
        for blk in f.blocks:
            insts = list(blk.instructions)
            out = []
            changed = False
            for inst in insts:
                si = inst.sync_info
                waits = list(si.on_wait) if si and si.on_wait else []
                if len(waits) > max_waits:
                    changed = True
                    n_split += 1
                    extra = waits[:-max_waits]
                    keep = waits[-max_waits:]
                    while extra:
                        chunk, extra = extra[:max_waits], extra[max_waits:]
                        cnt[0] += 1
                        nop = mybir.InstNoOp(
                            name=f"WSPLIT-{id(nc) % 100000}-{cnt[0]}",
                            ins=[], outs=[], engine=inst.engine)
                        nop.sync_info = mybir.SyncInfo(on_wait=chunk,
                                                       on_update=[])
                        out.append(nop)
                    inst.sync_info = mybir.SyncInfo(
                        on_wait=keep,
                        on_update=list(si.on_update) if si.on_update else [])
                out.append(inst)
            if changed:
                blk.instructions = out
    return n_split


# ------------------------------------------------------------- kernel build

def build_nc(k_iters=1):
    import concourse.bass as bass
    import concourse.mybir as mybir
    from concourse.tile import TileContext

    F32 = mybir.dt.float32
    BF16 = mybir.dt.bfloat16
    AF = mybir.ActivationFunctionType
    OP = mybir.AluOpType

    nc = bass.Bass()
    d = {}
    for name, shape, dt in [
        ('xT', [DIM, N], F32), ('yT', [DIM, N], F32),
        ('cosx', [P, N], F32), ('sinx', [P, N], F32),
        ('cosy', [P, N], F32), ('siny', [P, N], F32),
        ('wqA', [C, P, C, P], BF16), ('wqB', [C, P, C, P], BF16),
        ('wkA', [C, P, C, P], BF16), ('wkB', [C, P, C, P], BF16),
        ('wv', [P, C, DIM], BF16), ('wproj', [C, P, C, P], BF16),
        ('wcqA', [C, P, C, P], BF16), ('wcqB', [C, P, C, P], BF16),
        ('wckA', [C, P, C, P], BF16), ('wckB', [C, P, C, P], BF16),
        ('wcv', [P, C, DIM], BF16), ('wcproj', [C, P, C, P], BF16),
        ('wfc1', [HM, P, C, P], BF16), ('wfc2', [C, P, HM, P], BF16),
        ('bq', [P, C], F32), ('bqr', [P, C], F32),
        ('bk', [P, C], F32), ('bkr', [P, C], F32),
        ('bcq', [P, C], F32), ('bcqr', [P, C], F32),
        ('bck', [P, C], F32), ('bckr', [P, C], F32),
        ('bproj', [P, C], F32), ('bcproj', [P, C], F32),
        ('bfc1', [P, HM], F32), ('bfc2', [P, C], F32),
        ('bv_row', [1, DIM], F32), ('bcv_row', [1, DIM], F32),
        ('ones_bf', [P, P], BF16),
    ]:
        d[name] = nc.declare_dram_parameter(name, shape, dt, isOutput=False)
    out_d = nc.declare_dram_parameter('outT', [DIM, N], F32, isOutput=True)

    with TileContext(nc) as tc:
        with tc.tile_pool(name="const", bufs=1) as const, \
             tc.tile_pool(name="main", bufs=1) as main, \
             tc.tile_pool(name="work", bufs=2) as work, \
             tc.tile_pool(name="dscr", bufs=4, space="DRAM") as dscr, \
             tc.tile_pool(name="ps", bufs=4, space="PSUM") as psp:

            def body():
                # ---- constants ----
                cos_x = const.tile([P, N], F32, tag='cosx', name='cos_x')
                sin_x = const.tile([P, N], F32, tag='sinx', name='sin_x')
                cos_y = const.tile([P, N], F32, tag='cosy', name='cos_y')
                sin_y = const.tile([P, N], F32, tag='siny', name='sin_y')
                nc.sync.dma_start(cos_x[:], d['cosx'][:])
                nc.sync.dma_start(sin_x[:], d['sinx'][:])
                nc.sync.dma_start(cos_y[:], d['cosy'][:])
                nc.sync.dma_start(sin_y[:], d['siny'][:])
                ones_bf = const.tile([P, P], BF16, tag='ones', name='ones_bf')
                nc.sync.dma_start(ones_bf[:], d['ones_bf'][:])
                bias = {}
                for nm in ('bq', 'bqr', 'bk', 'bkr', 'bcq', 'bcqr', 'bck',
                           'bckr', 'bproj', 'bcproj', 'bfc2'):
                    bias[nm] = const.tile([P, C], F32, tag=nm, name=nm)
                    nc.sync.dma_start(bias[nm][:], d[nm][:])
                bias['bfc1'] = const.tile([P, HM], F32, tag='bfc1',
                                          name='bfc1')
                nc.sync.dma_start(bias['bfc1'][:], d['bfc1'][:])
                eps_t = const.tile([P, 1], F32, tag='eps', name='eps_t')
                nc.vector.memset(eps_t[:], EPS)
                bvrep = const.tile([P, DIM], F32, tag='bvrep', name='bvrep')
                nc.sync.dma_start(bvrep[:],
                                  d['bv_row'][:].to_broadcast((P, DIM)))
                bcvrep = const.tile([P, DIM], F32, tag='bcvrep', name='bcvrep')
                nc.sync.dma_start(bcvrep[:],
                                  d['bcv_row'][:].to_broadcast((P, DIM)))

                # ---- residual load ----
                xT = main.tile([P, C, N], F32, tag='xT', name='xT')
                nc.sync.dma_start(
                    xT[:], d['xT'][:].rearrange("(c p) t -> p c t", p=P))

                def w_cols(wd, colb, r0=0, rcnt=None):
                    """Pre-tiled weight block [128, rcnt, 128] bf16."""
                    cr = wd.shape[2]
                    if rcnt is None:
                        rcnt = cr
                    t = work.tile([P, rcnt, P], BF16, tag='wlhs', bufs=4,
                                  name='w_wlhs')
                    nc.sync.dma_start(t[:], wd[colb, :, r0:r0 + rcnt, :])
                    return t

                def layernorm(src, dst_tag):
                    """src [128, C, N] f32 -> bf16 normalized tile."""
                    xbf = main.tile([P, C, N], BF16, tag='expSC', name='xbf')
                    xsq = main.tile([P, C, N], BF16, tag='oTB', name='xsq')
                    for cc in range(C):
                        nc.vector.tensor_copy(xbf[:, cc, :], src[:, cc, :])
                        nc.vector.tensor_tensor(
                            xsq[:, cc, :], xbf[:, cc, :], xbf[:, cc, :],
                            OP.mult)
                    ps1 = psp.tile([P, N], F32, tag='ps', name='ps_sum')
                    ps2 = psp.tile([P, N], F32, tag='ps', name='ps_sumsq')
                    for cc in range(C):
                        for qh in range(2):
                            sl = slice(qh * 512, qh * 512 + 512)
                            nc.tensor.matmul(
                                ps1[:, sl], ones_bf[:], xbf[:, cc, sl],
                                start=(cc == 0), stop=(cc == C - 1))
                            nc.tensor.matmul(
                                ps2[:, sl], ones_bf[:], xsq[:, cc, sl],
                                start=(cc == 0), stop=(cc == C - 1))
                    m_rep = main.tile([P, N], F32, tag='s_m', name='m_rep')
                    nc.vector.tensor_scalar_mul(m_rep[:], ps1[:], 1.0 / DIM)
                    var = main.tile([P, N], F32, tag='s_var', name='var')
                    nc.vector.tensor_tensor(var[:], m_rep[:], m_rep[:],
                                            OP.mult)
                    nc.vector.scalar_tensor_tensor(
                        var[:], ps2[:], 1.0 / DIM, var[:],
                        OP.mult, OP.subtract)
                    nc.scalar.activation(var[:], var[:], AF.Ln, bias=eps_t[:])
                    rstd = main.tile([P, N], F32, tag='s_rstd', name='rstd')
                    nc.scalar.activation(rstd[:], var[:], AF.Exp, scale=-0.5)
                    h = main.tile([P, C, N], BF16, tag=dst_tag, name='h_out')
                    for cc in range(C):
                        tmpf = work.tile([P, N], F32, tag='lntmp', bufs=1,
                                         name='lntmp')
                        nc.vector.tensor_tensor(
                            tmpf[:], src[:, cc, :], m_rep[:], OP.subtract)
                        nc.vector.tensor_tensor(
                            h[:, cc, :], tmpf[:], rstd[:], OP.mult)
                    return h

                def build_vt(h, wv_d, bvr):
                    """V+ones lhsT tile [128, T, HEADS, 128] bf16."""
                    vt = main.tile([P, T, HEADS, P], BF16, tag='bigA',
                                   name='vt')
                    nc.gpsimd.memset(vt[:], 0.0)
                    nc.gpsimd.memset(vt[:, :, 0:HEADS:2, 64:65], 1.0)
                    nc.gpsimd.memset(vt[:, :, 1:HEADS:2, 63:64], 1.0)
                    wvt = main.tile([P, C, DIM], BF16, tag='wv_full',
                                    name='wvt')
                    nc.sync.dma_start(wvt[:], wv_d[:])
                    for tci in range(T):
                        pv = psp.tile([P, N], F32, tag='ps', name='pv')
                        for cc in range(C):
                            lhs = h[:, cc, tci * P:(tci + 1) * P]
                            nc.tensor.matmul(
                                pv[:, 0:512], lhs, wvt[:, cc, 0:512],
                                start=(cc == 0), stop=(cc == C - 1))
                            nc.tensor.matmul(
                                pv[:, 512:768], lhs, wvt[:, cc, 512:768],
                                start=(cc == 0), stop=(cc == C - 1))
                        pv_h = pv[:, 0:DIM].rearrange("p (h e) -> p h e", e=HD)
                        bv_h = bvr[:].rearrange("p (h e) -> p h e", e=HD)
                        nc.vector.tensor_tensor(
                            vt[:, tci, 0:HEADS:2, 0:64],
                            pv_h[:, 0:HEADS:2, :], bv_h[:, 0:HEADS:2, :],
                            OP.add)
                        nc.vector.tensor_tensor(
                            vt[:, tci, 1:HEADS:2, 64:128],
                            pv_h[:, 1:HEADS:2, :], bv_h[:, 1:HEADS:2, :],
                            OP.add)
                    return vt

                def qk_pair(h_src, wA, wB, bA, bAr, cost, sint, pi, tag):
                    """Roped Q^T or K^T pair-chunk [128, 1024] bf16."""
                    out = work.tile([P, N], BF16, tag=tag, name=f'{tag}_t')
                    wa = w_cols(wA, pi)
                    wb = w_cols(wB, pi)
                    for qh in range(2):
                        sl = slice(qh * 512, qh * 512 + 512)
                        pq = psp.tile([P, N], F32, tag='ps', name='pq')
                        for cc in range(C):
                            nc.tensor.matmul(
                                pq[:, 0:512], wa[:, cc, :], h_src[:, cc, sl],
                                start=(cc == 0), stop=(cc == C - 1))
                            nc.tensor.matmul(
                                pq[:, 512:1024], wb[:, cc, :],
                                h_src[:, cc, sl],
                                start=(cc == 0), stop=(cc == C - 1))
                        t1 = work.tile([P, 512], F32, tag='rtmp1', bufs=1,
                                       name='rt1')
                        t2 = work.tile([P, 512], F32, tag='rtmp2', bufs=1,
                                       name='rt2')
                        nc.vector.scalar_tensor_tensor(
                            t1[:], pq[:, 0:512], bA[:, pi:pi + 1],
                            cost[:, sl], OP.add, OP.mult)
                        nc.vector.scalar_tensor_tensor(
                            t2[:], pq[:, 512:1024], bAr[:, pi:pi + 1],
                            sint[:, sl], OP.add, OP.mult)
                        nc.vector.tensor_tensor(out[:, sl], t1[:], t2[:],
                                                OP.add)
                    return out

                def attention(h_q, h_k, vt, wgts, biases, cos_q, sin_q,
                              cos_k, sin_k, oT):
                    wqa, wqb, wka, wkb = wgts
                    bqa, bqra, bka, bkra = biases
                    for pi in range(NP):
                        qro = qk_pair(h_q, wqa, wqb, bqa, bqra,
                                      cos_q, sin_q, pi, 'qro')
                        kro = qk_pair(h_k, wka, wkb, bka, bkra,
                                      cos_k, sin_k, pi, 'kro')
                        for qh in range(2):
                            qsl = slice(qh * 512, qh * 512 + 512)
                            expS = main.tile([P, KC, 2, 512], BF16,
                                             tag='expSC', name='expS')
                            for kc in range(KC):
                                pss = psp.tile([P, N], F32, tag='ps',
                                               name='pss')
                                ksl = slice(kc * P, (kc + 1) * P)
                                nc.tensor.matmul(
                                    pss[:, 0:512], kro[0:64, ksl],
                                    qro[0:64, qsl], start=True, stop=True)
                                nc.tensor.matmul(
                                    pss[:, 512:1024], kro[64:128, ksl],
                                    qro[64:128, qsl], start=True, stop=True)
                                nc.scalar.activation(
                                    expS[:, kc, :, :], pss[:],
                                    AF.Exp, scale=float(HD) ** -0.5)
                            pav = psp.tile([P, N], F32, tag='ps', name='pav')
                            for kc in range(KC):
                                nc.tensor.matmul(
                                    pav[:, 0:512], vt[:, kc, 2 * pi, :],
                                    expS[:, kc, 0, :],
                                    start=(kc == 0), stop=(kc == KC - 1))
                                nc.tensor.matmul(
                                    pav[:, 512:1024],
                                    vt[:, kc, 2 * pi + 1, :],
                                    expS[:, kc, 1, :],
                                    start=(kc == 0), stop=(kc == KC - 1))
                            # denominators: row 64 (even head), row 63 (odd)
                            tln = main.tile([P, N], F32, tag='s_var',
                                            name='tln')
                            trec = main.tile([P, N], F32, tag='s_rstd',
                                             name='trec')
                            nc.scalar.activation(tln[:], pav[:], AF.Ln)
                            nc.scalar.activation(trec[:], tln[:], AF.Exp,
                                                 scale=-1.0)
                            scr = dscr.tile([2, N], F32, tag='scr',
                                            name='scr')
                            nc.sync.dma_start(scr[:], trec[63:65, :])
                            rep = work.tile([P, N], F32, tag='rep', bufs=1,
                                            name='rep')
                            nc.sync.dma_start(
                                rep[0:64, 0:512],
                                scr[1:2, 0:512].to_broadcast((64, 512)))
                            nc.sync.dma_start(
                                rep[64:128, 512:1024],
                                scr[0:1, 512:1024].to_broadcast((64, 512)))
                            nc.vector.tensor_tensor(
                                oT[0:64, pi, qsl], pav[0:64, 0:512],
                                rep[0:64, 0:512], OP.mult)
                            nc.vector.tensor_tensor(
                                oT[64:128, pi, qsl], pav[64:128, 512:1024],
                                rep[64:128, 512:1024], OP.mult)

                def proj_residual(oT, w_d, b_sb):
                    for m in range(C):
                        pp = psp.tile([P, N], F32, tag='ps', name='pp')
                        wp = w_cols(w_d, m)
                        for cc in range(C):
                            for qh in range(2):
                                sl = slice(qh * 512, qh * 512 + 512)
                                nc.tensor.matmul(
                                    pp[:, sl], wp[:, cc, :], oT[:, cc, sl],
                                    start=(cc == 0), stop=(cc == C - 1))
                        nc.vector.scalar_tensor_tensor(
                            xT[:, m, :], pp[:], b_sb[:, m:m + 1],
                            xT[:, m, :], OP.add, OP.add)

                # ================= self attention =================
                h1 = layernorm(xT, 'h')
                vt = build_vt(h1, d['wv'], bvrep)
                oT1 = main.tile([P, C, N], BF16, tag='oTB', name='oT1')
                attention(h1, h1, vt,
                          (d['wqA'], d['wqB'], d['wkA'], d['wkB']),
                          (bias['bq'], bias['bqr'], bias['bk'], bias['bkr']),
                          cos_x, sin_x, cos_x, sin_x, oT1)
                proj_residual(oT1, d['wproj'], bias['bproj'])

                # ================= cross attention =================
                yT = main.tile([P, C, N], F32, tag='bigA', name='yT')
                nc.sync.dma_start(
                    yT[:], d['yT'][:].rearrange("(c p) t -> p c t", p=P))
                y_ = layernorm(yT, 'y_')
                h2 = layernorm(xT, 'h')
                vtc = build_vt(y_, d['wcv'], bcvrep)
                oT2 = main.tile([P, C, N], BF16, tag='oTB', name='oT2')
                attention(h2, y_, vtc,
                          (d['wcqA'], d['wcqB'], d['wckA'], d['wckB']),
                          (bias['bcq'], bias['bcqr'], bias['bck'],
                           bias['bckr']),
                          cos_x, sin_x, cos_y, sin_y, oT2)
                proj_residual(oT2, d['wcproj'], bias['bcproj'])

                # ================= MLP (two hidden halves) =================
                h3 = layernorm(xT, 'h')
                outacc = main.tile([P, C, N], F32, tag='oTB', name='outacc')
                for half in range(2):
                    hidh = main.tile([P, HH, N], BF16, tag='bigA',
                                     name='hidh')
                    for hj in range(HH):
                        hm = half * HH + hj
                        ph = psp.tile([P, N], F32, tag='ps', name='ph')
                        wf = w_cols(d['wfc1'], hm)
                        for cc in range(C):
                            for qh in range(2):
                                sl = slice(qh * 512, qh * 512 + 512)
                                nc.tensor.matmul(
                                    ph[:, sl], wf[:, cc, :], h3[:, cc, sl],
                                    start=(cc == 0), stop=(cc == C - 1))
                        nc.scalar.activation(
                            hidh[:, hj, :], ph[:], AF.Gelu,
                            bias=bias['bfc1'][:, hm:hm + 1])
                    for m in range(C):
                        po = psp.tile([P, N], F32, tag='ps', name='po')
                        wf2 = w_cols(d['wfc2'], m, half * HH, HH)
                        for kj in range(HH):
                            for qh in range(2):
                                sl = slice(qh * 512, qh * 512 + 512)
                                nc.tensor.matmul(
                                    po[:, sl], wf2[:, kj, :], hidh[:, kj, sl],
                                    start=(kj == 0), stop=(kj == HH - 1))
                        if half == 0:
                            nc.vector.tensor_scalar_add(
                                outacc[:, m, :], po[:], bias['bfc2'][:, m:m + 1])
                        else:
                            nc.vector.tensor_tensor(
                                outacc[:, m, :], outacc[:, m, :], po[:],
                                OP.add)
                for m in range(C):
                    nc.vector.tensor_tensor(
                        xT[:, m, :], xT[:, m, :], outacc[:, m, :], OP.add)

                # ---- store ----
                nc.sync.dma_start(
                    out_d[:].rearrange("(c p) t -> p c t", p=P), xT[:])

            if k_iters > 1:
                with tc.For_i(0, k_iters, 1):
                    body()
            else:
                body()

    split_excess_waits(nc)
    return nc


# ------------------------------------------------------------------ driver

def kernel(**inputs):
    from concourse.bass_utils import run_bass_kernel_spmd
    per_core = prep_host(inputs)
    nc = build_nc(1)
    res = run_bass_kernel_spmd(nc, per_core, core_ids=list(range(B)))
    x_out = np.stack([np.ascontiguousarray(res.results[i]['outT'].T)
                      for i in range(B)])
    y = np.asarray(inputs['y'], dtype=np.float32)
    return (x_out.astype(np.float32), y)
